# revision 4
# baseline (speedup 1.0000x reference)
"""GraphSAGE (2-layer) + decoder + BCE loss on 8 TRN2 NeuronCores.

Strategy (graph/data parallel, per sharding hint):
  - Nodes sharded contiguously across 8 cores (6250 nodes/core).
  - Edges assigned to the core owning their *destination* node; each core
    aggregates messages for its own nodes only.
  - Gather h[src] with the SWDGE dma_gather instruction from a replicated
    node table in DRAM (x for layer 1; allgathered h1 for layer 2). Indices
    are int16, so the table is addressed as two halves (src < N/2 and
    src >= N/2); edges are grouped per (dst-bucket, half) on the host.
  - segment-sum by dst via a one-hot selection matrix (built on DVE with
    is_equal against an iota row) contracted on the TensorEngine into PSUM,
    accumulating over 128-edge chunks per 128-node destination bucket.
  - Mean scaling (1/deg) fused into the PSUM->SBUF evacuation on ACT.
  - Dense parts computed feature-major: out = Wl^T @ mean_fm + Wr^T @ h_fm,
    bias+relu fused into the PSUM evacuation on ACT.
  - One AllGather between the layers to replicate h1 (node-major).
  - Decoder + softplus row sums on device; final masked scalar reduction of
    the loss and shard concatenation happen on host ("unshard").
"""

import math
import sys

import numpy as np

sys.path.insert(0, "/opt/trn_rl_repo")

M = 8  # cores
GCH = 24  # chunks (of 128 edges) per dma_gather call

_CACHE = {}


# ---------------------------------------------------------------------------
# host-side preprocessing (index/layout only)
# ---------------------------------------------------------------------------
def _prep(x, xedge):
    N, D = x.shape
    E = xedge.shape[1]
    NL = N // M  # nodes per core
    NB = math.ceil(NL / 128)  # dst buckets per core
    NBP = NB * 128
    HALF = N // 2

    src = np.asarray(xedge[0], dtype=np.int64)
    dst = np.asarray(xedge[1], dtype=np.int64)

    deg = np.bincount(dst, minlength=N).astype(np.float32)
    rdeg = 1.0 / np.maximum(deg, 1.0)

    # order edges by (dst bucket, src half); within a (core,bucket,half)
    # group the order is irrelevant
    half = (src >= HALF).astype(np.int64)
    core = dst // NL
    ldst = dst - core * NL
    buck = ldst // 128
    gkey = (core * NB + buck) * 2 + half  # [2*M*NB] groups
    order = np.argsort(gkey, kind="stable")
    s_src, s_dst, s_key = src[order], dst[order], gkey[order]
    s_core = s_dst // NL
    s_ldst = s_dst - s_core * NL
    s_buck = s_ldst // 128
    s_half = s_key & 1

    cnt = np.bincount(s_key, minlength=2 * M * NB).reshape(M, NB, 2)
    # chunks per (bucket, half): identical across cores (SPMD)
    nch2 = np.ceil(cnt / 128.0).astype(np.int64).max(axis=0)  # [NB, 2]
    # ensure every bucket has >= 1 chunk so its PSUM bank gets written
    empty = nch2.sum(axis=1) == 0
    nch2[empty, 0] = 1
    NCHL, NCHH = int(nch2[:, 0].sum()), int(nch2[:, 1].sum())
    NCHT = NCHL + NCHH
    # global chunk column of each (bucket, half) group: lo block then hi block
    coff = np.zeros((NB, 2), np.int64)
    coff[:, 0] = np.cumsum(nch2[:, 0]) - nch2[:, 0]
    coff[:, 1] = NCHL + np.cumsum(nch2[:, 1]) - nch2[:, 1]

    starts = np.concatenate([[0], np.cumsum(cnt.ravel())]).astype(np.int64)
    pos = np.arange(E, dtype=np.int64) - starts[s_key]
    col = coff[s_buck, s_half] + pos // 128
    part = pos % 128

    eidx = np.zeros((M, 128, NCHT), np.int64)
    dstl = np.full((M, 128, NCHT), -1.0, np.float32)
    eidx[s_core, part, col] = s_src - s_half * HALF
    dstl[s_core, part, col] = (s_ldst - s_buck * 128).astype(np.float32)
    assert eidx.max() < 32768

    # wrap for dma_gather: slot (p, c) -> row p%16, col c*8 + p//16,
    # replicated over the 8 partition groups
    ew = (
        eidx.reshape(M, 8, 16, NCHT)
        .transpose(0, 2, 3, 1)
        .reshape(M, 16, NCHT * 8)
        .astype(np.int16)
    )
    eidx_w = np.tile(ew, (1, 8, 1))  # [M, 128, NCHT*8]

    rdeg_sb = np.ones((M, 128, NB), np.float32)
    nodes = np.arange(NL, dtype=np.int64)
    for k in range(M):
        rdeg_sb[k, nodes % 128, nodes // 128] = rdeg[k * NL + nodes]

    xfm = np.zeros((M, D, NBP), np.float32)
    for k in range(M):
        xfm[k, :, :NL] = np.asarray(x[k * NL : (k + 1) * NL], np.float32).T

    iota = np.broadcast_to(
        np.arange(128, dtype=np.float32)[None, :], (128, 128)
    ).copy()

    # gather-call ranges: split [0, NCHL) and [NCHL, NCHT) into GCH-sized runs
    calls = []  # (half, c0, nchunks)
    for h, lo, hi in ((0, 0, NCHL), (1, NCHL, NCHT)):
        c = lo
        while c < hi:
            w = min(GCH, hi - c)
            calls.append((h, c, w))
            c += w

    # chunk -> (call index, offset) map
    chunk_call = np.zeros(NCHT, np.int64)
    chunk_off = np.zeros(NCHT, np.int64)
    for ci, (h, c0, w) in enumerate(calls):
        chunk_call[c0 : c0 + w] = ci
        chunk_off[c0 : c0 + w] = np.arange(w)

    return dict(
        N=N, D=D, E=E, NL=NL, NB=NB, NBP=NBP, NCHT=NCHT, HALF=HALF,
        nch2=tuple(map(tuple, nch2)), coff=coff, calls=tuple(calls),
        chunk_call=chunk_call, chunk_off=chunk_off,
        eidx_w=eidx_w, dstl=dstl, rdeg_sb=rdeg_sb, xfm=xfm, iota=iota,
    )


# ---------------------------------------------------------------------------
# device kernel builder
# ---------------------------------------------------------------------------
def _build(meta):
    import concourse.bass as bass  # noqa: F401
    import concourse.tile as tile
    from concourse import bacc, mybir
    from concourse.library_config import mlp

    dt = mybir.dt
    f32 = dt.float32
    Alu = mybir.AluOpType
    Act = mybir.ActivationFunctionType

    N, D = meta["N"], meta["D"]
    NL, NB, NBP, NCHT = meta["NL"], meta["NB"], meta["NBP"], meta["NCHT"]
    HALF = meta["HALF"]
    nch2, coff, calls = meta["nch2"], meta["coff"], meta["calls"]
    chunk_call, chunk_off = meta["chunk_call"], meta["chunk_off"]
    H = D

    col_groups = []
    c = 0
    while c < NBP:
        w = min(512, NBP - c)
        col_groups.append((c, w))
        c += w

    nc = bacc.Bacc("TRN2", target_bir_lowering=False, debug=False, num_devices=M)

    xtab_d = nc.dram_tensor("xtab", [N, D], f32, kind="ExternalInput")
    xfm_d = nc.dram_tensor("xfm", [D, NBP], f32, kind="ExternalInput")
    eidx_d = nc.dram_tensor("eidx", [128, NCHT * 8], dt.int16, kind="ExternalInput")
    dstl_d = nc.dram_tensor("dstl", [128, NCHT], f32, kind="ExternalInput")
    rdeg_d = nc.dram_tensor("rdeg", [128, NB], f32, kind="ExternalInput")
    iota_d = nc.dram_tensor("iota", [128, 128], f32, kind="ExternalInput")
    ident_d = nc.dram_tensor("ident", [128, 128], f32, kind="ExternalInput")
    wdec_d = nc.dram_tensor("wdec", [D, H], f32, kind="ExternalInput")
    w_d = {}
    for nm in ("w1l", "w1r", "w2l", "w2r"):
        w_d[nm] = nc.dram_tensor(nm, [D, H], f32, kind="ExternalInput")
    b_d = {}
    for nm in ("b1l", "b2l"):
        b_d[nm] = nc.dram_tensor(nm, [128, 1], f32, kind="ExternalInput")

    h1shard = nc.dram_tensor("h1shard", [NL, D], f32)
    h1full = nc.dram_tensor("h1full", [N, D], f32, addr_space="Shared")

    emb_d = nc.dram_tensor("emb", [NL, D], f32, kind="ExternalOutput")
    lsum_d = nc.dram_tensor("lsum", [128, 2 * NB], f32, kind="ExternalOutput")

    with tile.TileContext(nc, num_cores=M) as tc:
        with (
            tc.tile_pool(name="const", bufs=1) as cpool,
            tc.tile_pool(name="fm", bufs=1) as fmpool,
            tc.tile_pool(name="msgs", bufs=3) as mpool,
            tc.tile_pool(name="oh", bufs=6) as ohpool,
            tc.tile_pool(name="nm", bufs=4) as nmpool,
            tc.tile_pool(name="psA", bufs=2, space="PSUM") as psA,
            tc.tile_pool(name="psT", bufs=2, space="PSUM") as psT,
            tc.tile_pool(name="psD", bufs=2, space="PSUM") as psD,
        ):
            nc.gpsimd.load_library(mlp)

            def load_const(name, dram, shape, dtype=f32):
                t = cpool.tile(shape, dtype, tag=name)
                nc.sync.dma_start(t[:], dram[:])
                return t

            eidx_t = load_const("eidx", eidx_d, [128, NCHT * 8], dt.int16)
            dstl_t = load_const("dstl", dstl_d, [128, NCHT])
            rdeg_t = load_const("rdeg", rdeg_d, [128, NB])
            iota_t = load_const("iota", iota_d, [128, 128])
            w_t = {nm: load_const(nm, w_d[nm], [D, H]) for nm in w_d}
            b_t = {nm: load_const(nm, b_d[nm], [128, 1]) for nm in b_d}
            wdec_t = load_const("wdec", wdec_d, [D, H])

            ident_t = load_const("ident", ident_d, [128, 128])

            xfm_t = fmpool.tile([D, NBP], f32, tag="fmA")
            nc.sync.dma_start(xfm_t[:], xfm_d[:])

            # ---- one SAGE layer ------------------------------------------
            def sage_layer(table, tfm_tile, wl, bl, wr, out_tag, relu):
                halves = (table[0:HALF, :], table[HALF:N, :])
                # issue gather calls lazily: tiles keyed by call index
                mtiles = {}

                def gather_call(ci):
                    if ci in mtiles:
                        return mtiles[ci]
                    h, c0, w = calls[ci]
                    t = mpool.tile([128, w, D], f32, tag="msgs")
                    nc.gpsimd.dma_gather(
                        t[:],
                        halves[h],
                        eidx_t[:, c0 * 8 : (c0 + w) * 8],
                        w * 128,
                        w * 128,
                        D,
                        single_packet=False,
                    )
                    mtiles[ci] = t
                    return t

                meanfm = fmpool.tile([D, NBP], f32, tag="fmC")
                for b in range(NB):
                    # global chunk columns of this bucket (lo then hi)
                    cols = []
                    for h in range(2):
                        c0 = int(coff[b][h])
                        cols.extend(range(c0, c0 + int(nch2[b][h])))
                    agg = psA.tile([128, 128], f32, tag="agg")
                    for j, col in enumerate(cols):
                        mt = gather_call(int(chunk_call[col]))
                        off = int(chunk_off[col])
                        oh = ohpool.tile([128, 128], f32, tag="oh")
                        nc.vector.tensor_tensor(
                            out=oh[:],
                            in0=dstl_t[:, col : col + 1].to_broadcast([128, 128]),
                            in1=iota_t[:],
                            op=Alu.is_equal,
                        )
                        nc.tensor.matmul(
                            out=agg[:],
                            lhsT=oh[:],
                            rhs=mt[:, off, :],
                            start=(j == 0),
                            stop=(j == len(cols) - 1),
                        )
                    mean_nm = nmpool.tile([128, 128], f32, tag="nm")
                    nc.scalar.activation(
                        out=mean_nm[:], in_=agg[:], func=Act.Copy,
                        scale=rdeg_t[:, b : b + 1],
                    )
                    tps = psT.tile([128, 128], f32, tag="tp")
                    nc.tensor.transpose(
                        out=tps[:], in_=mean_nm[:], identity=ident_t[:]
                    )
                    nc.vector.tensor_copy(
                        out=meanfm[:, b * 128 : (b + 1) * 128], in_=tps[:]
                    )
                outfm = fmpool.tile([D, NBP], f32, tag=out_tag)
                for c0, cw in col_groups:
                    ps = psD.tile([128, 512], f32, tag="dense")
                    nc.tensor.matmul(
                        out=ps[:, :cw], lhsT=wl[:], rhs=meanfm[:, c0 : c0 + cw],
                        start=True, stop=False,
                    )
                    nc.tensor.matmul(
                        out=ps[:, :cw], lhsT=wr[:], rhs=tfm_tile[:, c0 : c0 + cw],
                        start=False, stop=True,
                    )
                    nc.scalar.activation(
                        out=outfm[:, c0 : c0 + cw], in_=ps[:, :cw],
                        func=(Act.Relu if relu else Act.Identity),
                        bias=bl[:, 0:1],
                    )
                return outfm

            # ---- layer 1 --------------------------------------------------
            h1fm = sage_layer(
                xtab_d, xfm_t, w_t["w1l"], b_t["b1l"], w_t["w1r"],
                "fmB", relu=True,
            )

            # export node-major shard and allgather
            for b in range(NB):
                tps = psT.tile([128, 128], f32, tag="tp")
                nc.tensor.transpose(
                    out=tps[:], in_=h1fm[:, b * 128 : (b + 1) * 128],
                    identity=ident_t[:],
                )
                h1nm = nmpool.tile([128, 128], f32, tag="nm")
                nc.vector.tensor_copy(out=h1nm[:], in_=tps[:])
                rows = min(128, NL - b * 128)
                nc.sync.dma_start(
                    out=h1shard[b * 128 : b * 128 + rows, :], in_=h1nm[:rows, :]
                )
            nc.gpsimd.collective_compute(
                "AllGather",
                Alu.bypass,
                ins=[h1shard[:]],
                outs=[h1full[:]],
                replica_groups=[list(range(M))],
            )

            # ---- layer 2 --------------------------------------------------
            embfm = sage_layer(
                h1full, h1fm, w_t["w2l"], b_t["b2l"], w_t["w2r"],
                "fmA", relu=False,
            )

            # ---- decoder --------------------------------------------------
            dxfm = fmpool.tile([D, NBP], f32, tag="fmC")
            for c0, cw in col_groups:
                ps = psD.tile([128, 512], f32, tag="dense")
                nc.tensor.matmul(
                    out=ps[:, :cw], lhsT=wdec_t[:], rhs=embfm[:, c0 : c0 + cw],
                    start=True, stop=True,
                )
                nc.scalar.activation(
                    out=dxfm[:, c0 : c0 + cw], in_=ps[:, :cw], func=Act.Copy
                )

            # ---- emb export + loss row sums -------------------------------
            rs_t = cpool.tile([128, 2 * NB], f32, tag="rs")
            for b in range(NB):
                tps = psT.tile([128, 128], f32, tag="tp")
                nc.tensor.transpose(
                    out=tps[:], in_=embfm[:, b * 128 : (b + 1) * 128],
                    identity=ident_t[:],
                )
                emb_nm = nmpool.tile([128, 128], f32, tag="nm")
                nc.vector.tensor_copy(out=emb_nm[:], in_=tps[:])
                rows = min(128, NL - b * 128)
                nc.sync.dma_start(
                    out=emb_d[b * 128 : b * 128 + rows, :], in_=emb_nm[:rows, :]
                )

                tps2 = psT.tile([128, 128], f32, tag="tp")
                nc.tensor.transpose(
                    out=tps2[:], in_=dxfm[:, b * 128 : (b + 1) * 128],
                    identity=ident_t[:],
                )
                dx_nm = nmpool.tile([128, 128], f32, tag="dxnm")
                nc.scalar.activation(out=dx_nm[:], in_=tps2[:], func=Act.Copy)
                # softplus(-dx) = ln(1 + exp(-dx)); row-sum via ACT accumulator
                e_nm = nmpool.tile([128, 128], f32, tag="enm")
                nc.scalar.activation(
                    out=e_nm[:], in_=dx_nm[:], func=Act.Exp, scale=-1.0
                )
                sp_nm = nmpool.tile([128, 128], f32, tag="spnm")
                nc.scalar.activation(
                    out=sp_nm[:], in_=e_nm[:], func=Act.Ln, bias=1.0,
                    accum_out=rs_t[:, b : b + 1],
                )
                nc.vector.tensor_reduce(
                    out=rs_t[:, NB + b : NB + b + 1], in_=dx_nm[:],
                    axis=mybir.AxisListType.X, op=Alu.add,
                )
            nc.sync.dma_start(out=lsum_d[:], in_=rs_t[:])

    nc.compile()
    return nc


# ---------------------------------------------------------------------------
# public entry point
# ---------------------------------------------------------------------------
def _in_maps(meta, x, W1l, b1l, W1r, W2l, b2l, W2r, Wdec):
    bias_col = lambda v: np.asarray(v, np.float32).reshape(128, 1)
    maps = []
    for k in range(M):
        maps.append(
            {
                "xtab": x,
                "xfm": meta["xfm"][k],
                "eidx": meta["eidx_w"][k],
                "dstl": meta["dstl"][k],
                "rdeg": meta["rdeg_sb"][k],
                "iota": meta["iota"],
                "ident": np.eye(128, dtype=np.float32),
                "w1l": np.asarray(W1l, np.float32),
                "w1r": np.asarray(W1r, np.float32),
                "w2l": np.asarray(W2l, np.float32),
                "w2r": np.asarray(W2r, np.float32),
                "wdec": np.asarray(Wdec, np.float32),
                "b1l": bias_col(b1l),
                "b2l": bias_col(b2l),
            }
        )
    return maps


def _finish(meta, results, y):
    N, NL, NB, H = meta["N"], meta["NL"], meta["NB"], meta["D"]
    emb = np.concatenate([results[k]["emb"][:NL] for k in range(M)], axis=0)
    ym = np.asarray(y).astype(np.float32)
    n_pos = ym.sum()
    n_neg = N - n_pos
    nodes = np.arange(NL)
    p_i, b_i = nodes % 128, nodes // 128
    sp_sum = np.concatenate([results[k]["lsum"][p_i, b_i] for k in range(M)])
    dx_sum = np.concatenate(
        [results[k]["lsum"][p_i, NB + b_i] for k in range(M)]
    )
    l2 = float((sp_sum * ym).sum()) / (max(n_pos, 1.0) * H)
    l1 = float(((dx_sum + sp_sum) * (1.0 - ym)).sum()) / (max(n_neg, 1.0) * H)
    loss = np.float32(l1 + l2)
    return loss, emb


def kernel(x, xedge, y, pretrain, W1l, b1l, W1r, W2l, b2l, W2r, Wdec):
    from concourse.bass_utils import run_bass_kernel_spmd

    x = np.asarray(x, np.float32)
    meta = _prep(x, np.asarray(xedge))
    key = ("k", meta["N"], meta["D"], meta["NCHT"], meta["nch2"])
    if key not in _CACHE:
        _CACHE[key] = _build(meta)
    nc = _CACHE[key]

    maps = _in_maps(meta, x, W1l, b1l, W1r, W2l, b2l, W2r, Wdec)
    res = run_bass_kernel_spmd(nc, maps, core_ids=list(range(M)))
    loss, emb = _finish(meta, res.results, y)
    return (loss, emb)


# revision 5
# speedup vs baseline: 1.5330x; 1.5330x over previous
"""GraphSAGE (2-layer) + decoder + BCE loss on 8 TRN2 NeuronCores.

Strategy (graph/data parallel, per sharding hint):
  - Nodes sharded contiguously across 8 cores (6250 nodes/core).
  - Edges assigned to the core owning their *destination* node; each core
    aggregates messages for its own nodes only.
  - Gather h[src] with the SWDGE dma_gather instruction from a replicated
    node table in DRAM (x for layer 1; allgathered h1 for layer 2). Indices
    are int16, so the table is addressed as two halves (src < N/2 and
    src >= N/2); edges are grouped per (dst-bucket, half) on the host.
  - segment-sum by dst via a one-hot selection matrix (built on DVE with
    is_equal against an iota row) contracted on the TensorEngine into PSUM,
    accumulating over 128-edge chunks per 128-node destination bucket.
  - Mean scaling (1/deg) fused into the PSUM->SBUF evacuation on ACT.
  - Dense parts computed feature-major: out = Wl^T @ mean_fm + Wr^T @ h_fm,
    bias+relu fused into the PSUM evacuation on ACT.
  - One AllGather between the layers to replicate h1 (node-major).
  - Decoder + softplus row sums on device; final masked scalar reduction of
    the loss and shard concatenation happen on host ("unshard").
"""

import math
import sys

import numpy as np

sys.path.insert(0, "/opt/trn_rl_repo")

M = 8  # cores
GCH = 24  # chunks (of 128 edges) per dma_gather call

_CACHE = {}


# ---------------------------------------------------------------------------
# host-side preprocessing (index/layout only)
# ---------------------------------------------------------------------------
def _prep(x, xedge):
    N, D = x.shape
    E = xedge.shape[1]
    NL = N // M  # nodes per core
    NB = math.ceil(NL / 128)  # dst buckets per core
    NBP = NB * 128
    HALF = N // 2

    src = np.asarray(xedge[0], dtype=np.int64)
    dst = np.asarray(xedge[1], dtype=np.int64)

    deg = np.bincount(dst, minlength=N).astype(np.float32)
    rdeg = 1.0 / np.maximum(deg, 1.0)

    # order edges by (dst bucket, src half); within a (core,bucket,half)
    # group the order is irrelevant
    half = (src >= HALF).astype(np.int64)
    core = dst // NL
    ldst = dst - core * NL
    buck = ldst // 128
    gkey = (core * NB + buck) * 2 + half  # [2*M*NB] groups
    order = np.argsort(gkey, kind="stable")
    s_src, s_dst, s_key = src[order], dst[order], gkey[order]
    s_core = s_dst // NL
    s_ldst = s_dst - s_core * NL
    s_buck = s_ldst // 128
    s_half = s_key & 1

    cnt = np.bincount(s_key, minlength=2 * M * NB).reshape(M, NB, 2)
    # chunks per (bucket, half): identical across cores (SPMD)
    nch2 = np.ceil(cnt / 128.0).astype(np.int64).max(axis=0)  # [NB, 2]
    # ensure every bucket has >= 1 chunk so its PSUM bank gets written
    empty = nch2.sum(axis=1) == 0
    nch2[empty, 0] = 1
    NCHL, NCHH = int(nch2[:, 0].sum()), int(nch2[:, 1].sum())
    NCHT = NCHL + NCHH
    # global chunk column of each (bucket, half) group: lo block then hi block
    coff = np.zeros((NB, 2), np.int64)
    coff[:, 0] = np.cumsum(nch2[:, 0]) - nch2[:, 0]
    coff[:, 1] = NCHL + np.cumsum(nch2[:, 1]) - nch2[:, 1]

    starts = np.concatenate([[0], np.cumsum(cnt.ravel())]).astype(np.int64)
    pos = np.arange(E, dtype=np.int64) - starts[s_key]
    col = coff[s_buck, s_half] + pos // 128
    part = pos % 128

    eidx = np.zeros((M, 128, NCHT), np.int64)
    dstl = np.full((M, 128, NCHT), -1.0, np.float32)
    eidx[s_core, part, col] = s_src - s_half * HALF
    dstl[s_core, part, col] = (s_ldst - s_buck * 128).astype(np.float32)
    assert eidx.max() < 32768

    # wrap for dma_gather: slot (p, c) -> row p%16, col c*8 + p//16,
    # replicated over the 8 partition groups
    ew = (
        eidx.reshape(M, 8, 16, NCHT)
        .transpose(0, 2, 3, 1)
        .reshape(M, 16, NCHT * 8)
        .astype(np.int16)
    )
    eidx_w = np.tile(ew, (1, 8, 1))  # [M, 128, NCHT*8]

    rdeg_sb = np.ones((M, 128, NB), np.float32)
    nodes = np.arange(NL, dtype=np.int64)
    for k in range(M):
        rdeg_sb[k, nodes % 128, nodes // 128] = rdeg[k * NL + nodes]

    xfm = np.zeros((M, D, NBP), np.float32)
    for k in range(M):
        xfm[k, :, :NL] = np.asarray(x[k * NL : (k + 1) * NL], np.float32).T

    iota = np.broadcast_to(
        np.arange(128, dtype=np.float32)[None, :], (128, 128)
    ).copy()

    # gather-call ranges: split [0, NCHL) and [NCHL, NCHT) into GCH-sized runs
    calls = []  # (half, c0, nchunks)
    for h, lo, hi in ((0, 0, NCHL), (1, NCHL, NCHT)):
        c = lo
        while c < hi:
            w = min(GCH, hi - c)
            calls.append((h, c, w))
            c += w

    # chunk -> (call index, offset) map
    chunk_call = np.zeros(NCHT, np.int64)
    chunk_off = np.zeros(NCHT, np.int64)
    for ci, (h, c0, w) in enumerate(calls):
        chunk_call[c0 : c0 + w] = ci
        chunk_off[c0 : c0 + w] = np.arange(w)

    return dict(
        N=N, D=D, E=E, NL=NL, NB=NB, NBP=NBP, NCHT=NCHT, HALF=HALF,
        nch2=tuple(map(tuple, nch2)), coff=coff, calls=tuple(calls),
        chunk_call=chunk_call, chunk_off=chunk_off,
        eidx_w=eidx_w, dstl=dstl, rdeg_sb=rdeg_sb, xfm=xfm, iota=iota,
    )


# ---------------------------------------------------------------------------
# device kernel builder
# ---------------------------------------------------------------------------
def _build(meta):
    import concourse.bass as bass  # noqa: F401
    import concourse.tile as tile
    from concourse import bacc, mybir
    from concourse.library_config import mlp

    dt = mybir.dt
    f32 = dt.float32
    bf16 = dt.bfloat16
    Alu = mybir.AluOpType
    Act = mybir.ActivationFunctionType

    N, D = meta["N"], meta["D"]
    NL, NB, NBP, NCHT = meta["NL"], meta["NB"], meta["NBP"], meta["NCHT"]
    HALF = meta["HALF"]
    nch2, coff, calls = meta["nch2"], meta["coff"], meta["calls"]
    chunk_call, chunk_off = meta["chunk_call"], meta["chunk_off"]
    H = D

    col_groups = []
    c = 0
    while c < NBP:
        w = min(512, NBP - c)
        col_groups.append((c, w))
        c += w

    nc = bacc.Bacc("TRN2", target_bir_lowering=False, debug=False, num_devices=M)

    xtab_d = nc.dram_tensor("xtab", [N, D], bf16, kind="ExternalInput")
    xfm_d = nc.dram_tensor("xfm", [D, NBP], f32, kind="ExternalInput")
    eidx_d = nc.dram_tensor("eidx", [128, NCHT * 8], dt.int16, kind="ExternalInput")
    dstl_d = nc.dram_tensor("dstl", [128, NCHT], bf16, kind="ExternalInput")
    rdeg_d = nc.dram_tensor("rdeg", [128, NB], f32, kind="ExternalInput")
    iota_d = nc.dram_tensor("iota", [128, 128], bf16, kind="ExternalInput")
    ident_d = nc.dram_tensor("ident", [128, 128], f32, kind="ExternalInput")
    wdec_d = nc.dram_tensor("wdec", [D, H], f32, kind="ExternalInput")
    w_d = {}
    for nm in ("w1l", "w1r", "w2l", "w2r"):
        w_d[nm] = nc.dram_tensor(nm, [D, H], f32, kind="ExternalInput")
    b_d = {}
    for nm in ("b1l", "b2l"):
        b_d[nm] = nc.dram_tensor(nm, [128, 1], f32, kind="ExternalInput")

    h1shard = nc.dram_tensor("h1shard", [NL, D], bf16)
    h1full = nc.dram_tensor("h1full", [N, D], bf16, addr_space="Shared")

    emb_d = nc.dram_tensor("emb", [NL, D], f32, kind="ExternalOutput")
    lsum_d = nc.dram_tensor("lsum", [128, 2 * NB], f32, kind="ExternalOutput")

    with tile.TileContext(nc, num_cores=M) as tc:
        with (
            tc.tile_pool(name="const", bufs=1) as cpool,
            tc.tile_pool(name="fm", bufs=1) as fmpool,
            tc.tile_pool(name="msgs", bufs=3) as mpool,
            tc.tile_pool(name="oh", bufs=6) as ohpool,
            tc.tile_pool(name="nm", bufs=4) as nmpool,
            tc.tile_pool(name="psA", bufs=2, space="PSUM") as psA,
            tc.tile_pool(name="psT", bufs=2, space="PSUM") as psT,
            tc.tile_pool(name="psD", bufs=2, space="PSUM") as psD,
        ):
            nc.gpsimd.load_library(mlp)

            def load_const(name, dram, shape, dtype=f32):
                t = cpool.tile(shape, dtype, tag=name)
                nc.sync.dma_start(t[:], dram[:])
                return t

            eidx_t = load_const("eidx", eidx_d, [128, NCHT * 8], dt.int16)
            dstl_t = load_const("dstl", dstl_d, [128, NCHT], bf16)
            rdeg_t = load_const("rdeg", rdeg_d, [128, NB])
            iota_t = load_const("iota", iota_d, [128, 128], bf16)
            w_t = {nm: load_const(nm, w_d[nm], [D, H]) for nm in w_d}
            b_t = {nm: load_const(nm, b_d[nm], [128, 1]) for nm in b_d}
            wdec_t = load_const("wdec", wdec_d, [D, H])

            ident_t = load_const("ident", ident_d, [128, 128])

            xfm_t = fmpool.tile([D, NBP], f32, tag="fmA")
            nc.sync.dma_start(xfm_t[:], xfm_d[:])

            # ---- one SAGE layer ------------------------------------------
            def sage_layer(table, tfm_tile, wl, bl, wr, out_tag, relu):
                halves = (table[0:HALF, :], table[HALF:N, :])
                # issue gather calls lazily: tiles keyed by call index
                mtiles = {}

                def gather_call(ci):
                    if ci in mtiles:
                        return mtiles[ci]
                    h, c0, w = calls[ci]
                    t = mpool.tile([128, w, D], bf16, tag="msgs")
                    nc.gpsimd.dma_gather(
                        t[:],
                        halves[h],
                        eidx_t[:, c0 * 8 : (c0 + w) * 8],
                        w * 128,
                        w * 128,
                        D,
                        single_packet=False,
                    )
                    mtiles[ci] = t
                    return t

                meanfm = fmpool.tile([D, NBP], f32, tag="fmC")
                for b in range(NB):
                    # global chunk columns of this bucket (lo then hi)
                    cols = []
                    for h in range(2):
                        c0 = int(coff[b][h])
                        cols.extend(range(c0, c0 + int(nch2[b][h])))
                    agg = psA.tile([128, 128], f32, tag="agg")
                    for j, col in enumerate(cols):
                        mt = gather_call(int(chunk_call[col]))
                        off = int(chunk_off[col])
                        oh = ohpool.tile([128, 128], bf16, tag="oh")
                        nc.vector.tensor_tensor(
                            out=oh[:],
                            in0=dstl_t[:, col : col + 1].to_broadcast([128, 128]),
                            in1=iota_t[:],
                            op=Alu.is_equal,
                        )
                        nc.tensor.matmul(
                            out=agg[:],
                            lhsT=oh[:],
                            rhs=mt[:, off, :],
                            start=(j == 0),
                            stop=(j == len(cols) - 1),
                        )
                    mean_nm = nmpool.tile([128, 128], f32, tag="nm")
                    nc.scalar.activation(
                        out=mean_nm[:], in_=agg[:], func=Act.Copy,
                        scale=rdeg_t[:, b : b + 1],
                    )
                    tps = psT.tile([128, 128], f32, tag="tp")
                    nc.tensor.transpose(
                        out=tps[:], in_=mean_nm[:], identity=ident_t[:]
                    )
                    nc.vector.tensor_copy(
                        out=meanfm[:, b * 128 : (b + 1) * 128], in_=tps[:]
                    )
                outfm = fmpool.tile([D, NBP], f32, tag=out_tag)
                for c0, cw in col_groups:
                    ps = psD.tile([128, 512], f32, tag="dense")
                    nc.tensor.matmul(
                        out=ps[:, :cw], lhsT=wl[:], rhs=meanfm[:, c0 : c0 + cw],
                        start=True, stop=False,
                    )
                    nc.tensor.matmul(
                        out=ps[:, :cw], lhsT=wr[:], rhs=tfm_tile[:, c0 : c0 + cw],
                        start=False, stop=True,
                    )
                    nc.scalar.activation(
                        out=outfm[:, c0 : c0 + cw], in_=ps[:, :cw],
                        func=(Act.Relu if relu else Act.Identity),
                        bias=bl[:, 0:1],
                    )
                return outfm

            # ---- layer 1 --------------------------------------------------
            h1fm = sage_layer(
                xtab_d, xfm_t, w_t["w1l"], b_t["b1l"], w_t["w1r"],
                "fmB", relu=True,
            )

            # export node-major shard and allgather
            for b in range(NB):
                tps = psT.tile([128, 128], f32, tag="tp")
                nc.tensor.transpose(
                    out=tps[:], in_=h1fm[:, b * 128 : (b + 1) * 128],
                    identity=ident_t[:],
                )
                h1nm = nmpool.tile([128, 128], bf16, tag="h1nm")
                nc.vector.tensor_copy(out=h1nm[:], in_=tps[:])
                rows = min(128, NL - b * 128)
                nc.sync.dma_start(
                    out=h1shard[b * 128 : b * 128 + rows, :], in_=h1nm[:rows, :]
                )
            nc.gpsimd.collective_compute(
                "AllGather",
                Alu.bypass,
                ins=[h1shard[:]],
                outs=[h1full[:]],
                replica_groups=[list(range(M))],
            )

            # ---- layer 2 --------------------------------------------------
            embfm = sage_layer(
                h1full, h1fm, w_t["w2l"], b_t["b2l"], w_t["w2r"],
                "fmA", relu=False,
            )

            # ---- decoder --------------------------------------------------
            dxfm = fmpool.tile([D, NBP], f32, tag="fmC")
            for c0, cw in col_groups:
                ps = psD.tile([128, 512], f32, tag="dense")
                nc.tensor.matmul(
                    out=ps[:, :cw], lhsT=wdec_t[:], rhs=embfm[:, c0 : c0 + cw],
                    start=True, stop=True,
                )
                nc.scalar.activation(
                    out=dxfm[:, c0 : c0 + cw], in_=ps[:, :cw], func=Act.Copy
                )

            # ---- emb export + loss row sums -------------------------------
            rs_t = cpool.tile([128, 2 * NB], f32, tag="rs")
            for b in range(NB):
                tps = psT.tile([128, 128], f32, tag="tp")
                nc.tensor.transpose(
                    out=tps[:], in_=embfm[:, b * 128 : (b + 1) * 128],
                    identity=ident_t[:],
                )
                emb_nm = nmpool.tile([128, 128], f32, tag="nm")
                nc.vector.tensor_copy(out=emb_nm[:], in_=tps[:])
                rows = min(128, NL - b * 128)
                nc.sync.dma_start(
                    out=emb_d[b * 128 : b * 128 + rows, :], in_=emb_nm[:rows, :]
                )

                tps2 = psT.tile([128, 128], f32, tag="tp")
                nc.tensor.transpose(
                    out=tps2[:], in_=dxfm[:, b * 128 : (b + 1) * 128],
                    identity=ident_t[:],
                )
                dx_nm = nmpool.tile([128, 128], f32, tag="dxnm")
                nc.scalar.activation(out=dx_nm[:], in_=tps2[:], func=Act.Copy)
                # softplus(-dx) = ln(1 + exp(-dx)); row-sum via ACT accumulator
                e_nm = nmpool.tile([128, 128], f32, tag="enm")
                nc.scalar.activation(
                    out=e_nm[:], in_=dx_nm[:], func=Act.Exp, scale=-1.0
                )
                sp_nm = nmpool.tile([128, 128], f32, tag="spnm")
                nc.scalar.activation(
                    out=sp_nm[:], in_=e_nm[:], func=Act.Ln, bias=1.0,
                    accum_out=rs_t[:, b : b + 1],
                )
                nc.vector.tensor_reduce(
                    out=rs_t[:, NB + b : NB + b + 1], in_=dx_nm[:],
                    axis=mybir.AxisListType.X, op=Alu.add,
                )
            nc.sync.dma_start(out=lsum_d[:], in_=rs_t[:])

    nc.compile()
    return nc


# ---------------------------------------------------------------------------
# public entry point
# ---------------------------------------------------------------------------
def _in_maps(meta, x, W1l, b1l, W1r, W2l, b2l, W2r, Wdec):
    import ml_dtypes
    bf = ml_dtypes.bfloat16
    x_bf16 = np.asarray(x, np.float32).astype(bf)
    bias_col = lambda v: np.asarray(v, np.float32).reshape(128, 1)
    maps = []
    for k in range(M):
        maps.append(
            {
                "xtab": x_bf16,
                "xfm": meta["xfm"][k],
                "eidx": meta["eidx_w"][k],
                "dstl": meta["dstl"][k].astype(bf),
                "rdeg": meta["rdeg_sb"][k],
                "iota": meta["iota"].astype(bf),
                "ident": np.eye(128, dtype=np.float32),
                "w1l": np.asarray(W1l, np.float32),
                "w1r": np.asarray(W1r, np.float32),
                "w2l": np.asarray(W2l, np.float32),
                "w2r": np.asarray(W2r, np.float32),
                "wdec": np.asarray(Wdec, np.float32),
                "b1l": bias_col(b1l),
                "b2l": bias_col(b2l),
            }
        )
    return maps


def _finish(meta, results, y):
    N, NL, NB, H = meta["N"], meta["NL"], meta["NB"], meta["D"]
    emb = np.concatenate([results[k]["emb"][:NL] for k in range(M)], axis=0)
    ym = np.asarray(y).astype(np.float32)
    n_pos = ym.sum()
    n_neg = N - n_pos
    nodes = np.arange(NL)
    p_i, b_i = nodes % 128, nodes // 128
    sp_sum = np.concatenate([results[k]["lsum"][p_i, b_i] for k in range(M)])
    dx_sum = np.concatenate(
        [results[k]["lsum"][p_i, NB + b_i] for k in range(M)]
    )
    l2 = float((sp_sum * ym).sum()) / (max(n_pos, 1.0) * H)
    l1 = float(((dx_sum + sp_sum) * (1.0 - ym)).sum()) / (max(n_neg, 1.0) * H)
    loss = np.float32(l1 + l2)
    return loss, emb


def kernel(x, xedge, y, pretrain, W1l, b1l, W1r, W2l, b2l, W2r, Wdec):
    from concourse.bass_utils import run_bass_kernel_spmd

    x = np.asarray(x, np.float32)
    meta = _prep(x, np.asarray(xedge))
    key = ("k", meta["N"], meta["D"], meta["NCHT"], meta["nch2"])
    if key not in _CACHE:
        _CACHE[key] = _build(meta)
    nc = _CACHE[key]

    maps = _in_maps(meta, x, W1l, b1l, W1r, W2l, b2l, W2r, Wdec)
    res = run_bass_kernel_spmd(nc, maps, core_ids=list(range(M)))
    loss, emb = _finish(meta, res.results, y)
    return (loss, emb)
